# revision 34
# baseline (speedup 1.0000x reference)
# Multi-head causal attention (B=4, T=2048, D=1024, H=16, dk=64), fp32 in/out.
#
# Sharding: 8 cores = 4 batches x 2 head-groups (8 heads / 512 cols each).
# Each core computes a partial output  y_g @ wo_g  for its batch; the host
# sums the two head-group partials per batch and adds the constant row
# (bv @ wo + bo), which is exact because softmax rows sum to 1.
#
# v5 design (bf16 datapath + fp8 DoubleRow q/k projections):
#  - On TRN2 a DoubleRow fp8 matmul costs the same cycles/row as bf16 but
#    contracts 2x128 rows per instruction, so it halves cycle count only
#    for long contractions.  The only place that wins AND tolerates fp8
#    noise is the q/k projections (softmax damps the logit error; measured
#    ~1.1e-2 total vs the 2e-2 gate).  v/o/attention stay bf16.
#  - Softmax denominators ride inside the AV matmul: v_ext carries a ones
#    column at M-position 64 (M=65), so psy row 64 accumulates sum(exp)
#    and no separate denominator matmuls exist.  1/den rows collect in a
#    32-aligned ring (recb), get broadcast with one K=64 matmul per
#    head-pair wave, and normalize PSUM psy directly into packed (bf16).
#  - exp runs on ScalarE over [128, 2(heads), nw] PSUM score pairs; the
#    causal diagonal shrinks both the score matmuls and the exp range.
#  - Emission interleaves next-block projections and previous-block output
#    work (unit queue) into the attention stream; per-wave normalize tails
#    are deferred into the next wave so the in-order PE never stalls on
#    the DVE reciprocal chain.

from collections import deque

import numpy as np

B, T, D, H, DK = 4, 2048, 1024, 16, 64
NCORES = 8
G = 2               # head groups (tensor-parallel over heads)
C = D // G          # 512 columns per core = 8 heads
NH = C // DK        # heads per core = 8
NIB = T // 512      # 4 query blocks of 512
NJC = T // 128      # 16 key chunks of 128
WSC = 16.0          # host weight prescale (q/k path): keeps fp8 normal-range
SCALE = 1.0 / (8.0 * WSC * WSC)   # 1/sqrt(dk) / (16q * 16k)
VM = 65             # av stationary M: 64 v-dims + ones denominator col

MM_MODE = "v5"      # kept for test.py compatibility


def build_nc(mm_mode=MM_MODE, n_reps=1):
    from contextlib import ExitStack

    import concourse.bass as bass  # noqa: F401
    import concourse.mybir as mybir
    import concourse.tile as tile
    from concourse import bacc

    f32 = mybir.dt.float32
    bf16 = mybir.dt.bfloat16
    fp8 = mybir.dt.float8e4
    AF = mybir.ActivationFunctionType
    DR = mybir.MatmulPerfMode.DoubleRow

    nc = bacc.Bacc("TRN2", target_bir_lowering=False, debug=False,
                   num_devices=NCORES)

    x8_d = nc.dram_tensor("x8", [D, T], fp8, kind="ExternalInput").ap()
    xb_d = nc.dram_tensor("xb", [D, T], bf16, kind="ExternalInput").ap()
    wq_d = nc.dram_tensor("wq8", [D, C], fp8, kind="ExternalInput").ap()
    wk_d = nc.dram_tensor("wk8", [D, C], fp8, kind="ExternalInput").ap()
    wv_d = nc.dram_tensor("wvb", [D, C], bf16, kind="ExternalInput").ap()
    wo_d = nc.dram_tensor("wo", [C, D], bf16, kind="ExternalInput").ap()
    bq_d = nc.dram_tensor("bq", [C, 1], f32, kind="ExternalInput").ap()
    bk_d = nc.dram_tensor("bk", [C, 1], f32, kind="ExternalInput").ap()
    mtri_d = nc.dram_tensor("mtri2", [128, 256], f32,
                            kind="ExternalInput").ap()
    out_d = nc.dram_tensor("out", [T, D], f32, kind="ExternalOutput").ap()

    with tile.TileContext(nc) as tc, ExitStack() as st:
        pers = st.enter_context(tc.tile_pool(name="pers", bufs=1))
        sb = st.enter_context(tc.tile_pool(name="sb", bufs=1))
        ps = st.enter_context(tc.tile_pool(name="ps", bufs=1, space="PSUM"))

        mtri2 = pers.tile([128, 2, 128], f32, name="mtri2", tag="mtri2")
        bq_sb = pers.tile([128, 4], f32, name="bq_sb", tag="bq_sb")
        bk_sb = pers.tile([128, 4], f32, name="bk_sb", tag="bk_sb")
        sel2 = pers.tile([64, 128], bf16, name="sel2", tag="sel2")
        # 4-slot ring of 1/den rows; heads hi=0/1 live at partitions 0/32
        # (partition bases must be 32-aligned), other rows stay zero
        recb = pers.tile([64, 4, 512], bf16, name="recb", tag="recb")
        nc.sync.dma_start(mtri2[:], mtri_d.rearrange("p (j n) -> p j n", j=2))
        nc.sync.dma_start(bq_sb[:], bq_d.rearrange("(mb p) one -> p (mb one)",
                                                   p=128))
        nc.sync.dma_start(bk_sb[:], bk_d.rearrange("(mb p) one -> p (mb one)",
                                                   p=128))
        # sel2 @ recb broadcasts head hi's 1/den row to pb rows 64*hi..+64
        nc.gpsimd.memset(sel2[:], 0.0)
        nc.gpsimd.memset(sel2[0:1, 0:64], 1.0)
        nc.gpsimd.memset(sel2[32:33, 64:128], 1.0)
        nc.gpsimd.memset(recb[:], 0.0)
        wave_ctr = [0]

        def rep_state(r):
            """Per-rep tiles + weight DMAs (double-buffered rings)."""
            s = {}
            s["wq"] = sb.tile([128, 4, 2, C], fp8, name=f"{r}_wq", tag="wq",
                              bufs=2)
            s["wk"] = sb.tile([128, 4, 2, C], fp8, name=f"{r}_wk", tag="wk",
                              bufs=2)
            s["wv"] = sb.tile([128, 8, C], bf16, name=f"{r}_wv", tag="wv",
                              bufs=2)
            s["wo"] = sb.tile([128, 4, D], bf16, name=f"{r}_wo", tag="wo",
                              bufs=2)
            nc.sync.dma_start(s["wq"][:],
                              wq_d.rearrange("(t j p) m -> p t j m", t=4, j=2))
            nc.sync.dma_start(s["wk"][:],
                              wk_d.rearrange("(t j p) m -> p t j m", t=4, j=2))
            nc.sync.dma_start(s["wv"][:],
                              wv_d.rearrange("(dc p) m -> p dc m", p=128))
            nc.sync.dma_start(s["wo"][:],
                              wo_d.rearrange("(cc p) n -> p cc n", p=128))
            s["kT"] = [pers.tile([128, T], bf16, name=f"{r}_kT{cc}",
                                 tag=f"kT{cc}", bufs=2) for cc in range(4)]
            s["v"] = pers.tile([128, NJC, NH, VM], bf16, name=f"{r}_vx",
                               tag="v_ext", bufs=2)
            nc.gpsimd.memset(s["v"][:, :, :, 64:VM], 1.0)
            s["qT"] = {}
            return s

        def stage_a_units(r, s, ib):
            """Unit closures: x loads, q/k fp8-DR projections, v bf16."""
            loc = {}

            def ensure_x():
                if "x8" in loc:
                    return
                loc["x8"] = [sb.tile([128, 2, 512], fp8,
                                     name=f"{r}_x8_{ib}_{t}", tag=f"x8{t}",
                                     bufs=2) for t in range(4)]
                for t in range(4):
                    nc.sync.dma_start(
                        loc["x8"][t][:],
                        x8_d[256 * t:256 * (t + 1),
                             ib * 512:(ib + 1) * 512].rearrange(
                                 "(j p) n -> p j n", j=2))
                loc["xb"] = [sb.tile([128, 512], bf16,
                                     name=f"{r}_xb_{ib}_{dc}", tag=f"xb{dc}",
                                     bufs=2) for dc in range(8)]
                for dc in range(8):
                    nc.sync.dma_start(
                        loc["xb"][dc][:],
                        xb_d[128 * dc:128 * (dc + 1),
                             ib * 512:(ib + 1) * 512])

            units = []

            def u_load():
                ensure_x()
                s["qT"][ib] = [sb.tile([128, 512], bf16,
                                       name=f"{r}_qT_{ib}_{cc}",
                                       tag=f"qT{cc}", bufs=3)
                               for cc in range(4)]
            units.append(u_load)

            def u_qk(mb, w_sb, b_sb, dslice):
                def f():
                    ensure_x()
                    pq = ps.tile([128, 512], f32, name=f"{r}_p_{ib}_{mb}",
                                 tag="proj", bufs=2)
                    for t in range(4):
                        nc.tensor.matmul(
                            pq[:], w_sb[:, t, :, 128 * mb:128 * (mb + 1)],
                            loc["x8"][t][:], start=(t == 0), stop=(t == 3),
                            perf_mode=DR)
                    with nc.allow_low_precision(reason="16q/16k in bf16"):
                        nc.vector.tensor_scalar_add(dslice(), pq[:],
                                                    b_sb[:, mb:mb + 1])
                return f

            for cc in range(4):
                units.append(u_qk(
                    cc, s["wq"], bq_sb,
                    lambda cc=cc: s["qT"][ib][cc][:]))
                units.append(u_qk(
                    cc, s["wk"], bk_sb,
                    lambda cc=cc: s["kT"][cc][:, ib * 512:(ib + 1) * 512]))

            def u_v(isub):
                def f():
                    ensure_x()
                    pv = ps.tile([128, 512], f32, name=f"{r}_pv_{ib}_{isub}",
                                 tag="proj", bufs=2)
                    for dc in range(8):
                        nc.tensor.matmul(
                            pv[:],
                            loc["xb"][dc][:, isub * 128:(isub + 1) * 128],
                            s["wv"][:, dc, :], start=(dc == 0),
                            stop=(dc == 7))
                    with nc.allow_low_precision(reason="v in bf16"):
                        nc.vector.tensor_copy(
                            s["v"][:, ib * 4 + isub, :, 0:64],
                            pv[:].rearrange("p (h d) -> p h d", d=DK))
                return f
            for isub in range(4):
                units.append(u_v(isub))
            return units

        def stage_b(r, s, ib, queue):
            """Attention for query block ib; pops wave-tail closures (high
            priority) and `queue` units between steps so the PE/DVE stay fed
            while ScalarE (exp) streams."""
            njc = 4 * (ib + 1)
            packed = [sb.tile([128, 512], bf16, name=f"{r}_pk_{ib}_{cc}",
                              tag=f"packed{cc}", bufs=3) for cc in range(4)]
            qt = s["qT"][ib]
            nsteps = 4 * njc
            stride = max(1, nsteps // (len(queue) + 1)) if queue else nsteps
            step = 0
            tails = deque()

            def make_tail(w, psy):
                # per-head denominators (psy row 64) -> 1/den rows (recb
                # slot, partitions 0/32) -> broadcast via sel2 matmul ->
                # SBUF -> normalize both heads into packed[w]
                def f():
                    sl = wave_ctr[0] % 4
                    wave_ctr[0] += 1
                    with nc.allow_low_precision(reason="1/den in bf16"):
                        for hi in range(2):
                            nc.vector.reciprocal(
                                recb[32 * hi:32 * hi + 1, sl, :],
                                psy[hi][64:65, :])
                    pb = ps.tile([128, 512], f32, name=f"{r}_pb_{ib}_{w}",
                                 tag="proj", bufs=2)
                    nc.tensor.matmul(pb[:], sel2[:], recb[:, sl, :],
                                     start=True, stop=True)
                    pbs = sb.tile([128, 512], bf16, name=f"{r}_pbs_{ib}_{w}",
                                  tag="pbs", bufs=2)
                    with nc.allow_low_precision(reason="1/den in bf16"):
                        nc.vector.tensor_copy(pbs[:], pb[:])
                        for hi in range(2):
                            r0 = 64 * hi
                            nc.vector.tensor_mul(packed[w][r0:r0 + 64, :],
                                                 psy[hi][0:64, :],
                                                 pbs[r0:r0 + 64, :])
                return f

            for w in range(4):
                psy = [ps.tile([VM, 512], f32, name=f"{r}_psy_{ib}_{w}_{hi}",
                               tag="psy", bufs=2) for hi in range(2)]
                pend = deque()

                def emit_av(item, psy=psy):
                    jc, et, ij = item
                    for hi in range(2):
                        h = 2 * w + hi
                        nc.tensor.matmul(
                            psy[hi][0:VM, ij:512],
                            s["v"][:, jc, h, 0:VM],
                            et[:, hi, ij:512],
                            start=(jc == 0), stop=(jc == njc - 1))

                for jc in range(njc):
                    diag = jc >= 4 * ib
                    ij = 128 * (jc - 4 * ib) if diag else 0
                    pss = ps.tile([128, 2, 512], f32,
                                  name=f"{r}_pss_{ib}_{w}_{jc}",
                                  tag="pss", bufs=2)
                    for hi in range(2):
                        h0 = 64 * hi
                        nc.tensor.matmul(
                            pss[:, hi, ij:512],
                            s["kT"][w][h0:h0 + 64,
                                       jc * 128:(jc + 1) * 128],
                            qt[w][h0:h0 + 64, ij:512],
                            start=True, stop=True,
                            tile_position=(h0, 0))
                    if diag:
                        nc.vector.tensor_add(pss[:, :, ij:ij + 128],
                                             pss[:, :, ij:ij + 128],
                                             mtri2[:])
                    et = sb.tile([128, 2, 512], bf16,
                                 name=f"{r}_et_{ib}_{w}_{jc}", tag="et",
                                 bufs=8)
                    nc.scalar.activation(et[:, :, ij:512],
                                         pss[:, :, ij:512], AF.Exp,
                                         scale=SCALE)
                    pend.append((jc, et, ij))
                    if len(pend) >= 3:
                        emit_av(pend.popleft())
                    step += 1
                    if tails:
                        tails.popleft()()
                    elif queue and step % stride == 0:
                        queue.popleft()()
                while pend:
                    emit_av(pend.popleft())
                tails.append(make_tail(w, psy))
            while tails:
                tails.popleft()()
            while queue:
                queue.popleft()()
            return packed

        def o_units(r, ib, s, packed):
            units = []

            def u_o(isub):
                def f():
                    osb = sb.tile([128, D], f32, name=f"{r}_osb_{ib}_{isub}",
                                  tag="osb", bufs=2)
                    for nb in range(2):
                        pso = ps.tile([128, 512], f32,
                                      name=f"{r}_pso_{ib}_{isub}_{nb}",
                                      tag="proj", bufs=2)
                        for cc in range(4):
                            nc.tensor.matmul(
                                pso[:],
                                packed[cc][:, isub * 128:(isub + 1) * 128],
                                s["wo"][:, cc, nb * 512:(nb + 1) * 512],
                                start=(cc == 0), stop=(cc == 3))
                        nc.vector.tensor_copy(osb[:, nb * 512:(nb + 1) * 512],
                                              pso[:])
                    row0 = (ib * 4 + isub) * 128
                    nc.sync.dma_start(out_d[row0:row0 + 128, :], osb[:])
                return f
            for isub in range(4):
                units.append(u_o(isub))
            return units

        # flat pipelined schedule across reps:
        #   B0(r): A2(r) | B1(r): O3(r-1)+O0(r)+A3(r) | B2(r): O1(r)+A0(r+1)
        #   B3(r): O2(r)+A1(r+1)
        states = {}

        def get_state(r):
            if r not in states:
                states[r] = rep_state(r)
            return states[r]

        s0 = get_state(0)
        for unit in stage_a_units(0, s0, 0):
            unit()
        for unit in stage_a_units(0, s0, 1):
            unit()
        prev_o3 = []
        for r in range(n_reps):
            s = get_state(r)
            a2 = stage_a_units(r, s, 2)
            pk0 = stage_b(r, s, 0, deque(a2))
            o0 = o_units(r, 0, s, pk0)
            a3 = stage_a_units(r, s, 3)
            pk1 = stage_b(r, s, 1, deque(_ilv(prev_o3 + o0, a3)))
            o1 = o_units(r, 1, s, pk1)
            nxt = []
            if r + 1 < n_reps:
                sn = get_state(r + 1)
                nxt = [stage_a_units(r + 1, sn, 0),
                       stage_a_units(r + 1, sn, 1)]
            pk2 = stage_b(r, s, 2, deque(_ilv(o1, nxt[0] if nxt else [])))
            o2 = o_units(r, 2, s, pk2)
            pk3 = stage_b(r, s, 3, deque(_ilv(o2, nxt[1] if nxt else [])))
            prev_o3 = o_units(r, 3, s, pk3)
        for unit in prev_o3:
            unit()

    nc.compile()
    return nc


def _ilv(a, b):
    """Interleave two unit lists, a first."""
    out = []
    la, lb = list(a), list(b)
    n = max(len(la), len(lb))
    for i in range(n):
        if i < len(la):
            out.append(la[i])
        if i < len(lb):
            out.append(lb[i])
    return out


def make_in_maps(x, wq, bq, wk, bk, wv, bv, wo, bo):
    import ml_dtypes
    fp8 = ml_dtypes.float8_e4m3
    bf16 = ml_dtypes.bfloat16

    jj = np.arange(128)[:, None]
    ii = np.arange(128)[None, :]
    tri = np.where(ii < jj, -1e9, 0.0).astype(np.float32)   # mask q < key
    mtri2 = np.concatenate([tri, tri], axis=1)              # both head slots

    x = np.asarray(x, np.float32)
    in_maps = []
    for c in range(NCORES):
        b, g = c // G, c % G
        cs = slice(g * C, (g + 1) * C)
        xT = np.ascontiguousarray(x[b].T)
        in_maps.append({
            "x8": xT.astype(fp8),
            "xb": xT.astype(bf16),
            "wq8": (np.asarray(wq[:, cs], np.float32) * WSC).astype(fp8),
            "wk8": (np.asarray(wk[:, cs], np.float32) * WSC).astype(fp8),
            "wvb": np.asarray(wv[:, cs], np.float32).astype(bf16),
            "wo": np.ascontiguousarray(wo[cs, :], dtype=np.float32).astype(bf16),
            "bq": np.ascontiguousarray(
                np.asarray(bq[cs], np.float32).reshape(C, 1) * WSC),
            "bk": np.ascontiguousarray(
                np.asarray(bk[cs], np.float32).reshape(C, 1) * WSC),
            "mtri2": mtri2,
        })
    return in_maps


_NC_CACHE = {}


def _get_nc(mm_mode=MM_MODE):
    if mm_mode not in _NC_CACHE:
        _NC_CACHE[mm_mode] = build_nc(mm_mode)
    return _NC_CACHE[mm_mode]


def kernel(x, mask, wq, bq, wk, bk, wv, bv, wo, bo, _trace=False, _results=None):
    from concourse.bass_utils import run_bass_kernel_spmd

    nc = _get_nc()
    in_maps = make_in_maps(np.asarray(x), np.asarray(wq), np.asarray(bq),
                           np.asarray(wk), np.asarray(bk), np.asarray(wv),
                           np.asarray(bv), np.asarray(wo), np.asarray(bo))
    res = run_bass_kernel_spmd(nc, in_maps, core_ids=list(range(NCORES)),
                               trace=_trace)
    if _results is not None:
        _results.append(res)
    # constant row: y += bv (since attn rows sum to 1)  =>  out += bv@wo + bo
    row_const = (np.asarray(bv, np.float64) @ np.asarray(wo, np.float64)
                 + np.asarray(bo, np.float64)).astype(np.float32)
    out = np.empty((B, T, D), dtype=np.float32)
    for b in range(B):
        out[b] = (res.results[2 * b]["out"] + res.results[2 * b + 1]["out"]
                  + row_const)
    return out


# revision 36
# speedup vs baseline: 1.3902x; 1.3902x over previous
# Multi-head causal attention (B=4, T=2048, D=1024, H=16, dk=64), fp32 in/out.
#
# Sharding: 8 cores = 4 batches x 2 head-groups (8 heads / 512 cols each).
# Each core computes a partial output  y_g @ wo_g  for its batch; the host
# sums the two head-group partials per batch and adds the constant row
# (bv @ wo + bo), which is exact because softmax rows sum to 1.
#
# v5 design (bf16 datapath + fp8 DoubleRow q/k projections):
#  - On TRN2 a DoubleRow fp8 matmul costs the same cycles/row as bf16 but
#    contracts 2x128 rows per instruction, so it halves cycle count only
#    for long contractions.  The only place that wins AND tolerates fp8
#    noise is the q/k projections (softmax damps the logit error; measured
#    ~1.1e-2 total vs the 2e-2 gate).  v/o/attention stay bf16.
#  - Softmax denominators ride inside the AV matmul: v_ext carries a ones
#    column at M-position 64 (M=65), so psy row 64 accumulates sum(exp)
#    and no separate denominator matmuls exist.  1/den rows collect in a
#    32-aligned ring (recb), get broadcast with one K=64 matmul per
#    head-pair wave, and normalize PSUM psy directly into packed (bf16).
#  - exp runs on ScalarE over [128, 2(heads), nw] PSUM score pairs; the
#    causal diagonal shrinks both the score matmuls and the exp range.
#  - Emission interleaves next-block projections and previous-block output
#    work (unit queue) into the attention stream; per-wave normalize tails
#    are deferred into the next wave so the in-order PE never stalls on
#    the DVE reciprocal chain.

from collections import deque

import numpy as np

B, T, D, H, DK = 4, 2048, 1024, 16, 64
NCORES = 8
G = 2               # head groups (tensor-parallel over heads)
C = D // G          # 512 columns per core = 8 heads
NH = C // DK        # heads per core = 8
NIB = T // 512      # 4 query blocks of 512
NJC = T // 128      # 16 key chunks of 128
WSC = 16.0          # host weight prescale (q/k path): keeps fp8 normal-range
SCALE = 1.0 / (8.0 * WSC * WSC)   # 1/sqrt(dk) / (16q * 16k)
VM = 65             # av stationary M: 64 v-dims + ones denominator col

MM_MODE = "v5"      # kept for test.py compatibility


def build_nc(mm_mode=MM_MODE, n_reps=1):
    from contextlib import ExitStack

    import concourse.bass as bass  # noqa: F401
    import concourse.mybir as mybir
    import concourse.tile as tile
    from concourse import bacc

    f32 = mybir.dt.float32
    bf16 = mybir.dt.bfloat16
    fp8 = mybir.dt.float8e4
    AF = mybir.ActivationFunctionType
    DR = mybir.MatmulPerfMode.DoubleRow

    nc = bacc.Bacc("TRN2", target_bir_lowering=False, debug=False,
                   num_devices=NCORES)

    x8_d = nc.dram_tensor("x8", [D, T], fp8, kind="ExternalInput").ap()
    xb_d = nc.dram_tensor("xb", [D, T], bf16, kind="ExternalInput").ap()
    wq_d = nc.dram_tensor("wq8", [D, C], fp8, kind="ExternalInput").ap()
    wk_d = nc.dram_tensor("wk8", [D, C], fp8, kind="ExternalInput").ap()
    wv_d = nc.dram_tensor("wvb", [D, C], bf16, kind="ExternalInput").ap()
    wo_d = nc.dram_tensor("wo", [C, D], bf16, kind="ExternalInput").ap()
    bq_d = nc.dram_tensor("bq", [C, 1], f32, kind="ExternalInput").ap()
    bk_d = nc.dram_tensor("bk", [C, 1], f32, kind="ExternalInput").ap()
    mtri_d = nc.dram_tensor("mtri2", [128, 256], f32,
                            kind="ExternalInput").ap()
    out_d = nc.dram_tensor("out", [T, D], f32, kind="ExternalOutput").ap()

    with tile.TileContext(nc) as tc, ExitStack() as st:
        pers = st.enter_context(tc.tile_pool(name="pers", bufs=1))
        sb = st.enter_context(tc.tile_pool(name="sb", bufs=1))
        ps = st.enter_context(tc.tile_pool(name="ps", bufs=1, space="PSUM"))

        mtri2 = pers.tile([128, 2, 128], f32, name="mtri2", tag="mtri2")
        bq_sb = pers.tile([128, 4], f32, name="bq_sb", tag="bq_sb")
        bk_sb = pers.tile([128, 4], f32, name="bk_sb", tag="bk_sb")
        sel2 = pers.tile([64, 128], bf16, name="sel2", tag="sel2")
        # 4-slot ring of 1/den rows; heads hi=0/1 live at partitions 0/32
        # (partition bases must be 32-aligned), other rows stay zero
        recb = pers.tile([64, 4, 512], bf16, name="recb", tag="recb")
        nc.sync.dma_start(mtri2[:], mtri_d.rearrange("p (j n) -> p j n", j=2))
        nc.sync.dma_start(bq_sb[:], bq_d.rearrange("(mb p) one -> p (mb one)",
                                                   p=128))
        nc.sync.dma_start(bk_sb[:], bk_d.rearrange("(mb p) one -> p (mb one)",
                                                   p=128))
        # sel2 @ recb broadcasts head hi's 1/den row to pb rows 64*hi..+64
        nc.gpsimd.memset(sel2[:], 0.0)
        nc.gpsimd.memset(sel2[0:1, 0:64], 1.0)
        nc.gpsimd.memset(sel2[32:33, 64:128], 1.0)
        nc.gpsimd.memset(recb[:], 0.0)
        wave_ctr = [0]

        def rep_state(r):
            """Per-rep tiles + weight DMAs (double-buffered rings)."""
            s = {}
            s["wq"] = sb.tile([128, 4, 2, C], fp8, name=f"{r}_wq", tag="wq",
                              bufs=2)
            s["wk"] = sb.tile([128, 4, 2, C], fp8, name=f"{r}_wk", tag="wk",
                              bufs=2)
            s["wv"] = sb.tile([128, 8, C], bf16, name=f"{r}_wv", tag="wv",
                              bufs=2)
            s["wo"] = sb.tile([128, 4, D], bf16, name=f"{r}_wo", tag="wo",
                              bufs=2)
            nc.sync.dma_start(s["wq"][:],
                              wq_d.rearrange("(t j p) m -> p t j m", t=4, j=2))
            nc.sync.dma_start(s["wk"][:],
                              wk_d.rearrange("(t j p) m -> p t j m", t=4, j=2))
            nc.sync.dma_start(s["wv"][:],
                              wv_d.rearrange("(dc p) m -> p dc m", p=128))
            nc.sync.dma_start(s["wo"][:],
                              wo_d.rearrange("(cc p) n -> p cc n", p=128))
            s["kT"] = [pers.tile([128, T], bf16, name=f"{r}_kT{cc}",
                                 tag=f"kT{cc}", bufs=2) for cc in range(4)]
            s["v"] = pers.tile([128, NJC, NH, VM], bf16, name=f"{r}_vx",
                               tag="v_ext", bufs=2)
            nc.gpsimd.memset(s["v"][:, :, :, 64:VM], 1.0)
            s["qT"] = {}
            return s

        def stage_a_units(r, s, ib):
            """Unit closures: x loads, q/k fp8-DR projections, v bf16."""
            loc = {}

            def ensure_x():
                if "x8" in loc:
                    return
                loc["x8"] = [sb.tile([128, 2, 512], fp8,
                                     name=f"{r}_x8_{ib}_{t}", tag=f"x8{t}",
                                     bufs=2) for t in range(4)]
                for t in range(4):
                    nc.sync.dma_start(
                        loc["x8"][t][:],
                        x8_d[256 * t:256 * (t + 1),
                             ib * 512:(ib + 1) * 512].rearrange(
                                 "(j p) n -> p j n", j=2))
                loc["xb"] = [sb.tile([128, 512], bf16,
                                     name=f"{r}_xb_{ib}_{dc}", tag=f"xb{dc}",
                                     bufs=2) for dc in range(8)]
                for dc in range(8):
                    nc.sync.dma_start(
                        loc["xb"][dc][:],
                        xb_d[128 * dc:128 * (dc + 1),
                             ib * 512:(ib + 1) * 512])

            units = []

            def u_load():
                ensure_x()
                s["qT"][ib] = [sb.tile([128, 512], bf16,
                                       name=f"{r}_qT_{ib}_{cc}",
                                       tag=f"qT{cc}", bufs=3)
                               for cc in range(4)]
            units.append(u_load)

            def u_qk(mb, w_sb, b_sb, dslice):
                def f():
                    ensure_x()
                    pq = ps.tile([128, 512], f32, name=f"{r}_p_{ib}_{mb}",
                                 tag="proj", bufs=2)
                    for t in range(4):
                        nc.tensor.matmul(
                            pq[:], w_sb[:, t, :, 128 * mb:128 * (mb + 1)],
                            loc["x8"][t][:], start=(t == 0), stop=(t == 3),
                            perf_mode=DR)
                    with nc.allow_low_precision(reason="16q/16k in bf16"):
                        nc.vector.tensor_scalar_add(dslice(), pq[:],
                                                    b_sb[:, mb:mb + 1])
                return f

            for cc in range(4):
                units.append(u_qk(
                    cc, s["wq"], bq_sb,
                    lambda cc=cc: s["qT"][ib][cc][:]))
                units.append(u_qk(
                    cc, s["wk"], bk_sb,
                    lambda cc=cc: s["kT"][cc][:, ib * 512:(ib + 1) * 512]))

            def u_v(isub):
                def f():
                    ensure_x()
                    pv = ps.tile([128, 512], f32, name=f"{r}_pv_{ib}_{isub}",
                                 tag="proj", bufs=2)
                    for dc in range(8):
                        nc.tensor.matmul(
                            pv[:],
                            loc["xb"][dc][:, isub * 128:(isub + 1) * 128],
                            s["wv"][:, dc, :], start=(dc == 0),
                            stop=(dc == 7))
                    with nc.allow_low_precision(reason="v in bf16"):
                        nc.vector.tensor_copy(
                            s["v"][:, ib * 4 + isub, :, 0:64],
                            pv[:].rearrange("p (h d) -> p h d", d=DK))
                return f
            for isub in range(4):
                units.append(u_v(isub))
            return units

        def stage_b(r, s, ib, queue):
            """Attention for query block ib; pops wave-tail closures (high
            priority) and `queue` units between steps so the PE/DVE stay fed
            while ScalarE (exp) streams."""
            njc = 4 * (ib + 1)
            packed = [sb.tile([128, 512], bf16, name=f"{r}_pk_{ib}_{cc}",
                              tag=f"packed{cc}", bufs=3) for cc in range(4)]
            qt = s["qT"][ib]
            nsteps = 4 * njc
            stride = max(1, nsteps // (len(queue) + 1)) if queue else nsteps
            step = 0
            tails = deque()

            def make_tail(w, psy):
                # per-head denominators (psy row 64) -> 1/den rows (recb
                # slot, partitions 0/32) -> broadcast via sel2 matmul ->
                # SBUF -> normalize both heads into packed[w]
                def f():
                    sl = wave_ctr[0] % 4
                    wave_ctr[0] += 1
                    with nc.allow_low_precision(reason="1/den in bf16"):
                        for hi in range(2):
                            nc.vector.reciprocal(
                                recb[32 * hi:32 * hi + 1, sl, :],
                                psy[hi][64:65, :])
                    pb = ps.tile([128, 512], f32, name=f"{r}_pb_{ib}_{w}",
                                 tag="proj", bufs=2)
                    nc.tensor.matmul(pb[:], sel2[:], recb[:, sl, :],
                                     start=True, stop=True)
                    pbs = sb.tile([128, 512], bf16, name=f"{r}_pbs_{ib}_{w}",
                                  tag="pbs", bufs=2)
                    with nc.allow_low_precision(reason="1/den in bf16"):
                        nc.vector.tensor_copy(pbs[:], pb[:])
                        for hi in range(2):
                            r0 = 64 * hi
                            nc.vector.tensor_mul(packed[w][r0:r0 + 64, :],
                                                 psy[hi][0:64, :],
                                                 pbs[r0:r0 + 64, :])
                return f

            for w in range(4):
                psy = [ps.tile([VM, 512], f32, name=f"{r}_psy_{ib}_{w}_{hi}",
                               tag="psy", bufs=2) for hi in range(2)]
                pend = deque()

                def emit_av(item, psy=psy):
                    jc, et, ij = item
                    for hi in range(2):
                        h = 2 * w + hi
                        nc.tensor.matmul(
                            psy[hi][0:VM, ij:512],
                            s["v"][:, jc, h, 0:VM],
                            et[:, hi, ij:512],
                            start=(jc == 0), stop=(jc == njc - 1))

                for jc in range(njc):
                    diag = jc >= 4 * ib
                    ij = 128 * (jc - 4 * ib) if diag else 0
                    pss = ps.tile([128, 2, 512], f32,
                                  name=f"{r}_pss_{ib}_{w}_{jc}",
                                  tag="pss", bufs=2)
                    for hi in range(2):
                        h0 = 64 * hi
                        nc.tensor.matmul(
                            pss[:, hi, ij:512],
                            s["kT"][w][h0:h0 + 64,
                                       jc * 128:(jc + 1) * 128],
                            qt[w][h0:h0 + 64, ij:512],
                            start=True, stop=True,
                            tile_position=(h0, 0))
                    if diag:
                        nc.vector.tensor_add(pss[:, :, ij:ij + 128],
                                             pss[:, :, ij:ij + 128],
                                             mtri2[:])
                    et = sb.tile([128, 2, 512], bf16,
                                 name=f"{r}_et_{ib}_{w}_{jc}", tag="et",
                                 bufs=8)
                    nc.scalar.activation(et[:, :, ij:512],
                                         pss[:, :, ij:512], AF.Exp,
                                         scale=SCALE)
                    pend.append((jc, et, ij))
                    if len(pend) >= 3:
                        emit_av(pend.popleft())
                    step += 1
                    if tails:
                        tails.popleft()()
                    elif queue and step % stride == 0:
                        queue.popleft()()
                while pend:
                    emit_av(pend.popleft())
                tails.append(make_tail(w, psy))
            while tails:
                tails.popleft()()
            while queue:
                queue.popleft()()
            return packed

        def o_units(r, ib, s, packed):
            units = []

            def u_o(isub):
                def f():
                    osb = sb.tile([128, D], f32, name=f"{r}_osb_{ib}_{isub}",
                                  tag="osb", bufs=2)
                    for nb in range(2):
                        pso = ps.tile([128, 512], f32,
                                      name=f"{r}_pso_{ib}_{isub}_{nb}",
                                      tag="proj", bufs=2)
                        for cc in range(4):
                            nc.tensor.matmul(
                                pso[:],
                                packed[cc][:, isub * 128:(isub + 1) * 128],
                                s["wo"][:, cc, nb * 512:(nb + 1) * 512],
                                start=(cc == 0), stop=(cc == 3))
                        nc.vector.tensor_copy(osb[:, nb * 512:(nb + 1) * 512],
                                              pso[:])
                    row0 = (ib * 4 + isub) * 128
                    nc.sync.dma_start(out_d[row0:row0 + 128, :], osb[:])
                return f
            for isub in range(4):
                units.append(u_o(isub))
            return units

        # flat pipelined schedule across reps:
        #   B0(r): A2(r) | B1(r): O3(r-1)+O0(r)+A3(r) | B2(r): O1(r)+A0(r+1)
        #   B3(r): O2(r)+A1(r+1)
        states = {}

        def get_state(r):
            if r not in states:
                states[r] = rep_state(r)
            return states[r]

        s0 = get_state(0)
        for unit in stage_a_units(0, s0, 0):
            unit()
        for unit in stage_a_units(0, s0, 1):
            unit()
        prev_o3 = []
        for r in range(n_reps):
            s = get_state(r)
            a2 = stage_a_units(r, s, 2)
            pk0 = stage_b(r, s, 0, deque(a2))
            o0 = o_units(r, 0, s, pk0)
            a3 = stage_a_units(r, s, 3)
            pk1 = stage_b(r, s, 1, deque(_ilv(prev_o3 + o0, a3)))
            o1 = o_units(r, 1, s, pk1)
            nxt = []
            if r + 1 < n_reps:
                sn = get_state(r + 1)
                nxt = [stage_a_units(r + 1, sn, 0),
                       stage_a_units(r + 1, sn, 1)]
            pk2 = stage_b(r, s, 2, deque(_ilv(o1, nxt[0] if nxt else [])))
            o2 = o_units(r, 2, s, pk2)
            pk3 = stage_b(r, s, 3, deque(_ilv(o2, nxt[1] if nxt else [])))
            prev_o3 = o_units(r, 3, s, pk3)
        for unit in prev_o3:
            unit()

    nc.compile()
    return nc


def _ilv(a, b):
    """Interleave two unit lists, a first."""
    out = []
    la, lb = list(a), list(b)
    n = max(len(la), len(lb))
    for i in range(n):
        if i < len(la):
            out.append(la[i])
        if i < len(lb):
            out.append(lb[i])
    return out


def make_in_maps(x, wq, bq, wk, bk, wv, bv, wo, bo):
    import ml_dtypes
    fp8 = ml_dtypes.float8_e4m3
    bf16 = ml_dtypes.bfloat16

    jj = np.arange(128)[:, None]
    ii = np.arange(128)[None, :]
    tri = np.where(ii < jj, -1e9, 0.0).astype(np.float32)   # mask q < key
    mtri2 = np.concatenate([tri, tri], axis=1)              # both head slots

    x = np.asarray(x, np.float32)
    in_maps = []
    for c in range(NCORES):
        b, g = c // G, c % G
        cs = slice(g * C, (g + 1) * C)
        xT = np.ascontiguousarray(x[b].T)
        in_maps.append({
            "x8": xT.astype(fp8),
            "xb": xT.astype(bf16),
            "wq8": (np.asarray(wq[:, cs], np.float32) * WSC).astype(fp8),
            "wk8": (np.asarray(wk[:, cs], np.float32) * WSC).astype(fp8),
            "wvb": np.asarray(wv[:, cs], np.float32).astype(bf16),
            "wo": np.ascontiguousarray(wo[cs, :], dtype=np.float32).astype(bf16),
            "bq": np.ascontiguousarray(
                np.asarray(bq[cs], np.float32).reshape(C, 1) * WSC),
            "bk": np.ascontiguousarray(
                np.asarray(bk[cs], np.float32).reshape(C, 1) * WSC),
            "mtri2": mtri2,
        })
    return in_maps


_NC_CACHE = {}


def _get_nc(mm_mode=MM_MODE):
    if mm_mode not in _NC_CACHE:
        _NC_CACHE[mm_mode] = build_nc(mm_mode)
    return _NC_CACHE[mm_mode]


def kernel(x, mask, wq, bq, wk, bk, wv, bv, wo, bo, _trace=False, _results=None):
    from concourse.bass_utils import run_bass_kernel_spmd

    nc = _get_nc()
    in_maps = make_in_maps(np.asarray(x), np.asarray(wq), np.asarray(bq),
                           np.asarray(wk), np.asarray(bk), np.asarray(wv),
                           np.asarray(bv), np.asarray(wo), np.asarray(bo))
    res = run_bass_kernel_spmd(nc, in_maps, core_ids=list(range(NCORES)),
                               trace=_trace)
    if _results is not None:
        _results.append(res)
    # constant row: y += bv (since attn rows sum to 1)  =>  out += bv@wo + bo
    row_const = (np.asarray(bv, np.float64) @ np.asarray(wo, np.float64)
                 + np.asarray(bo, np.float64)).astype(np.float32)
    out = np.empty((B, T, D), dtype=np.float32)
    for b in range(B):
        out[b] = (res.results[2 * b]["out"] + res.results[2 * b + 1]["out"]
                  + row_const)
    return out
